# revision 26
# baseline (speedup 1.0000x reference)
"""Trainium2 Bass kernel for CGNN message-passing ODE (nn_CGNN_51333449121989).

Math: the reference integrates the affine ODE z' = diag(sigmoid(alpha))*0.5*(A z - z) + x0
with RK4 (4 steps, dt=0.25) from z0 = x0, where x0 = [x @ m1_w + m1_b, zeros].
Since each RK4 step is the affine map z <- P(M) z + Q(M) x0 with
M = diag(a)*0.5*(A - I), the final state is an exact degree-16 polynomial
R(M) x0, evaluated here by 16 Horner iterations:
    y <- a05 * (A y - y) + r_k * x0      (a05 = 0.5*sigmoid(alpha))
Feature columns H..2H-1 of the state are identically zero (columns evolve
independently and start/force at zero), so the working state is [N, H].

Distribution: 1D node partition over 8 cores (6250 rows each, padded to
6272 = 49*128).  Each core owns the edges whose src falls in its row range.
Per iteration each core:
  - dma_gather's y[dst] rows (256B each) from a full HBM replica of y
    (int16 gather indices => the replica is split in two halves; edges are
    routed into a "low" and a "high" gather stream),
  - segment-sums messages into psum per 128-row block with PE matmuls:
    psum[128 rows, H] += onehot(src_local)^T @ msg, where the per-chunk
    onehot-with-weights matrix W[e, r] = (r == src_local[e]) * w_e is built
    on the vector engine from an iota tile via a dual-op tensor_scalar,
  - applies the Horner update, publishes its shard and AllGathers the next
    replica.
The 16 iterations are flat python-unrolled: collectives cannot execute
inside a For_i hardware loop on this toolchain, and dma_scatter_add loses
colliding updates, so the onehot-matmul segment-sum stays.

Transfer layer: the per-call wall time is dominated by host->device upload
through the PJRT tunnel (~40MB/s for incompressible bytes, large per-array
fixed cost).  Countermeasures:
  - ONE flat int16 blob per core holds every input (10-bit fixed-point x
    unpacked on device via integer shift/mask DVE ops; bf16 tables; int8
    src rows; int16 gather indices uploaded once at 16 rows and replicated
    to 128 on device; iota/identity built on device), output returned as
    bf16;
  - jax persistent compilation cache so warm calls skip the walrus backend
    compile, plus a per-instance memo of the BIR JSON serialization that
    the jax lowering otherwise recomputes every call.
Host-side work is limited to sharding/packing: edge bucketing + padding to a
core-uniform chunk structure, gather-index wrapping, and input layout.
"""

import os
import sys

sys.path.insert(0, "/opt/trn_rl_repo")

from dataclasses import dataclass

import numpy as np
import ml_dtypes


def _setup_jax_compilation_cache():
    """Persistent XLA compilation cache: warm run_bass_kernel_spmd calls hit
    the cache instead of re-running the (~2s) walrus backend compile."""
    try:
        import jax

        jax.config.update("jax_compilation_cache_dir", "/tmp/jax_comp_cache")
        jax.config.update("jax_persistent_cache_min_compile_time_secs", 0)
        jax.config.update("jax_persistent_cache_min_entry_size_bytes", -1)
    except Exception:
        pass


_setup_jax_compilation_cache()


# ---------------------------------------------------------------- constants
@dataclass(frozen=True)
class Cfg:
    N: int = 50000          # nodes
    E: int = 600000         # edges
    F: int = 500            # input features
    H: int = 64             # hidden (ODE state width)
    C: int = 40             # classes
    NCORES: int = 8
    NITER: int = 16         # Horner iterations (degree-16 polynomial, exact)
    DT: float = 0.25        # T / STEPS from the reference
    GCH: int = 48           # gather-group size in chunks (48*128 idx per call)

    @property
    def NSH(self):          # true rows per core
        return self.N // self.NCORES

    @property
    def BLOCKS(self):       # 128-row blocks per core
        return (self.NSH + 127) // 128

    @property
    def NLOC(self):         # padded rows per core
        return self.BLOCKS * 128

    @property
    def NREP(self):         # replica rows
        return self.NCORES * self.NLOC

    @property
    def HALF(self):         # low/high split of replica rows (int16 gather idx)
        return self.NREP // 2

    @property
    def KP(self):           # encoder contraction dim padded (F + bias row)
        return ((self.F + 1 + 127) // 128) * 128


def horner_coeffs(cfg: Cfg) -> np.ndarray:
    """Coefficients r_0..r_16 of the exact RK4 polynomial R(M)."""
    dt = cfg.DT
    deg = max(cfg.NITER, 16)
    P = np.zeros(deg + 1)
    Q = np.zeros(deg + 1)
    P[0] = 1.0
    fact = 1.0
    for j in range(1, 5):
        fact *= j
        P[j] = dt**j / fact
        Q[j - 1] = dt**j / fact

    def pmul(a, b):
        out = np.zeros(2 * deg + 1)
        for i in range(deg + 1):
            if a[i]:
                out[i : i + deg + 1] += a[i] * b
        return out[: deg + 1]

    P2 = pmul(P, P)
    P3 = pmul(P2, P)
    P4 = pmul(P3, P)
    S = P3 + P2 + P
    S[0] += 1.0
    R = P4 + pmul(S, Q)
    return R


# ------------------------------------------------------------ tile patch
def _patch_tile_drain():
    """This toolchain's walrus rejects instructions with several sem waits;
    split TileContext's exit-drain waits across single-wait nops."""
    import concourse.tile as tile
    from concourse.vector_clock import ScopedClock
    from bass_rust import VectorClock

    if getattr(tile.TileContext, "_drain_patched", False):
        return

    def _drain_and_barrier(self, tick_clock, wait_clock):
        gc = tick_clock.global_clock
        scoped = ScopedClock({None: gc})
        for scope, vc in scoped.items():
            procs = [i for i in range(len(vc)) if vc[i] > 0]
            for p in procs:
                pvc = VectorClock()
                pvc.require_at_least(p, vc[p])
                nop = self.nc.sync.nop(nofuse=True, hint="drain_split")
                wait_clock.add_sem_waits(nop.ins, ScopedClock({scope: pvc}))
        self.nc.sync.drain()
        self.nc.all_engine_barrier()
        assert self.sems is not None
        popped = self.nc._tile_sem_poison_stack.pop()
        assert popped is self._sem_poison
        self.nc.clear_and_free_semaphores(list(self.sems.allocated().values()))
        self.nc.all_engine_barrier()

    tile.TileContext._drain_and_barrier = _drain_and_barrier
    tile.TileContext._drain_patched = True


# ------------------------------------------------------------ host prep
def _align(v, a=128):
    return (v + a - 1) // a * a


@dataclass
class Plan:
    # uniform chunk structure
    nch_l: int
    nch_h: int
    cbs: np.ndarray           # [BLOCKS, 2] chunks per (block, stream)
    ngrp_l: int
    ngrp_h: int
    # 10-bit x quantization scale (global)
    xscale: float
    # blob section offsets (int16 elements), core-uniform
    off: dict
    blob_len: int
    # per-core packed tensors
    in_maps: list


def build_plan(cfg: Cfg, inputs: dict) -> Plan:
    x = np.asarray(inputs["x"], np.float32)
    ew = np.asarray(inputs["edge_w"], np.float32)
    src = np.asarray(inputs["edge_src"], np.int64)
    dst = np.asarray(inputs["edge_dst"], np.int64)
    m1w = np.asarray(inputs["m1_w"], np.float32)
    m1b = np.asarray(inputs["m1_b"], np.float32)
    alpha = np.asarray(inputs["alpha_train"], np.float32)
    m2w = np.asarray(inputs["m2_w"], np.float32)
    m2b = np.asarray(inputs["m2_b"], np.float32)
    bf16 = ml_dtypes.bfloat16

    NC, NSH, NLOC, BLOCKS = cfg.NCORES, cfg.NSH, cfg.NLOC, cfg.BLOCKS
    HALF, GCH, KP = cfg.HALF, cfg.GCH, cfg.KP

    owner = src // NSH
    owner = np.minimum(owner, NC - 1)
    src_loc = src - owner * NSH
    downer = dst // NSH
    downer = np.minimum(downer, NC - 1)
    dpos = downer * NLOC + (dst - downer * NSH)   # replica row of dst
    stream = (dpos >= HALF).astype(np.int64)      # 0 = low, 1 = high
    block = src_loc // 128
    srow = src_loc % 128                          # row within block

    # ---- per-(core, block, stream) edge buckets
    counts = np.zeros((NC, BLOCKS, 2), np.int64)
    np.add.at(counts, (owner, block, stream), 1)
    cbs = np.ceil(counts.max(axis=0) / 128).astype(np.int64)   # [BLOCKS, 2]
    cbs[:, 0] = np.maximum(cbs[:, 0], 1)    # every block needs >=1 chunk
    nch_l = int(cbs[:, 0].sum())
    nch_h = int(cbs[:, 1].sum())
    nch = nch_l + nch_h
    ngrp_l = (nch_l + GCH - 1) // GCH
    ngrp_h = (nch_h + GCH - 1) // GCH
    nil = ngrp_l * GCH * 8        # idx columns per 16-row band, low
    nih = ngrp_h * GCH * 8

    # chunk column offsets per (block, stream); stream H columns offset by nch_l
    off_l = np.concatenate([[0], np.cumsum(cbs[:, 0])])
    off_h = np.concatenate([[0], np.cumsum(cbs[:, 1])])

    # ---- 10-bit fixed point for x: q = round(x*s) in [-511, 511]
    xscale = 511.0 / max(float(np.abs(x).max()), 1e-6)

    # ---- blob layout (int16 elements, 128-aligned sections, core-uniform)
    off = {}
    pos = 0
    def sect(name, n):
        nonlocal pos
        off[name] = pos
        pos += _align(n)
    sect("xtb10", BLOCKS * KP * 80)         # packed 10-bit [BLOCKS, KP, 5, 16]
    sect("m1w", KP * cfg.H)                 # bf16 [KP, H]
    sect("m2w", (cfg.H + 1) * cfg.C * 2)    # f32  [H+1, C]
    sect("alpha", 128 * BLOCKS)             # bf16 [128, BLOCKS]
    sect("srct", 64 * nch)                  # int8 [128, nch]
    sect("wt", 128 * nch)                   # bf16 [128, nch]
    sect("idxl", 16 * nil)                  # int16 [16, nil]
    sect("idxh", 16 * nih)                  # int16 [16, nih]
    blob_len = _align(pos, 1024)

    m1w_aug = np.zeros((KP, cfg.H), np.float32)
    m1w_aug[: cfg.F] = m1w
    m1w_aug[cfg.F] = m1b
    m1w_b = m1w_aug.astype(bf16)
    m2w_aug = np.zeros((cfg.H + 1, cfg.C), np.float32)
    m2w_aug[: cfg.H] = m2w
    m2w_aug[cfg.H] = m2b

    def put(blob, name, arr):
        a16 = np.ascontiguousarray(arr).reshape(-1).view(np.int16)
        blob[off[name] : off[name] + a16.size] = a16

    def pack10(vals):
        """vals [R, 128] signed ints in [-512, 511] -> [R, 5, 16] uint16."""
        q = (vals.astype(np.int32) + 512).astype(np.uint32).reshape(-1, 16, 8)
        q0, q1, q2, q3 = q[:, :, 0], q[:, :, 1], q[:, :, 2], q[:, :, 3]
        q4, q5, q6, q7 = q[:, :, 4], q[:, :, 5], q[:, :, 6], q[:, :, 7]
        w0 = (q0 << 6) | (q1 >> 4)
        w1 = ((q1 & 0xF) << 12) | (q2 << 2) | (q3 >> 8)
        w2 = ((q3 & 0xFF) << 8) | (q4 >> 2)
        w3 = ((q4 & 0x3) << 14) | (q5 << 4) | (q6 >> 6)
        w4 = ((q6 & 0x3F) << 10) | q7
        return np.stack([w0, w1, w2, w3, w4], axis=1).astype(np.uint16)

    # sort edges per core by (stream, block, dst) for gather locality
    in_maps = []
    for c in range(NC):
        sel = owner == c
        eb, es, er, ed, ewc = (
            block[sel], stream[sel], srow[sel], dpos[sel], ew[sel])

        src_tab = np.zeros((128, nch), np.int8)
        w_tab = np.zeros((128, nch), bf16)
        idx_l = np.zeros(nch_l * 128, np.int64)
        idx_h = np.zeros(nch_h * 128, np.int64)

        for s, (idx_arr, offc, base_col) in enumerate(
            [(idx_l, off_l, 0), (idx_h, off_h, nch_l)]
        ):
            m = es == s
            b_s, r_s, d_s, w_s = eb[m], er[m], ed[m], ewc[m]
            order = np.lexsort((d_s, b_s))
            b_s, r_s, d_s, w_s = b_s[order], r_s[order], d_s[order], w_s[order]
            # place edges of block b into its chunk range [offc[b], offc[b+1])
            starts = np.searchsorted(b_s, np.arange(BLOCKS))
            ends = np.searchsorted(b_s, np.arange(BLOCKS), side="right")
            for b in range(BLOCKS):
                n_edges = ends[b] - starts[b]
                pos0 = offc[b] * 128
                sl = slice(starts[b], ends[b])
                idx_arr[pos0 : pos0 + n_edges] = d_s[sl] - s * HALF
                cols = np.arange(n_edges) // 128 + offc[b]
                rows = np.arange(n_edges) % 128
                src_tab[rows, base_col + cols] = r_s[sl].astype(np.int8)
                w_tab[rows, base_col + cols] = w_s[sl]
                # padding edges keep w=0 / idx=0 / src_row=0

        def wrap_idx(idx_arr, ngrp):
            n = ngrp * GCH * 128
            full = np.zeros(n, np.int64)
            full[: len(idx_arr)] = idx_arr
            return full.reshape(-1, 16).T.astype(np.int16)    # [16, n/16]

        # encoder inputs: block-tiled x^T with bias row, 10-bit packed
        rows = slice(c * NSH, (c + 1) * NSH)
        xsh = np.zeros((NLOC, cfg.F), np.float32)
        xsh[:NSH] = x[rows]
        xtb_q = np.zeros((BLOCKS, KP, 128), np.int32)
        for b in range(BLOCKS):
            xtb_q[b, : cfg.F, :] = np.round(
                xsh[b * 128 : (b + 1) * 128].T * xscale).astype(np.int32)
            xtb_q[b, cfg.F, :] = int(round(xscale))     # bias row ~ 1.0
        xtb_p = pack10(xtb_q.reshape(-1, 128)).reshape(BLOCKS, KP, 5, 16)

        al = np.zeros(NLOC, np.float32)
        al[:NSH] = alpha[rows]
        alpha_b = al.reshape(BLOCKS, 128).T.astype(bf16)    # [128, BLOCKS]

        blob = np.zeros(blob_len, np.int16)
        put(blob, "xtb10", xtb_p)
        put(blob, "m1w", m1w_b)
        put(blob, "m2w", m2w_aug)
        put(blob, "alpha", alpha_b)
        put(blob, "srct", src_tab)
        put(blob, "wt", w_tab)
        put(blob, "idxl", wrap_idx(idx_l, ngrp_l))
        put(blob, "idxh", wrap_idx(idx_h, ngrp_h))
        in_maps.append(dict(blob=blob))

    return Plan(nch_l, nch_h, np.asarray(cbs), ngrp_l, ngrp_h,
                xscale, off, blob_len, in_maps)


# ------------------------------------------------------------ device program
def build_program(cfg: Cfg, plan: Plan, rcoef: np.ndarray,
                  timing_mode: bool = False, phases: str = "ehda",
                  reps=(1, 1, 1)):
    RG, RM, RU = reps   # timing: repeat gathers / matmuls / updates
    """timing_mode: single-core variant for TimelineSim (collectives replaced
    by a local DMA of the same local traffic)."""
    import concourse.bacc as bacc
    import concourse.mybir as mybir
    import concourse.tile as tile

    _patch_tile_drain()

    NC, H, BLOCKS, NLOC, NREP, HALF = (
        cfg.NCORES, cfg.H, cfg.BLOCKS, cfg.NLOC, cfg.NREP, cfg.HALF)
    GCH, KP = cfg.GCH, cfg.KP
    KCH = KP // 128
    f32 = mybir.dt.float32
    bf16 = mybir.dt.bfloat16
    i16 = mybir.dt.int16
    i8 = mybir.dt.int8
    Alu = mybir.AluOpType
    nch = plan.nch_l + plan.nch_h
    nil = plan.ngrp_l * GCH * 8
    nih = plan.ngrp_h * GCH * 8
    O = plan.off

    nc = bacc.Bacc("TRN2", target_bir_lowering=False, debug=False,
                   num_devices=1 if timing_mode else NC)

    def allgather(ins, outs):
        if "a" not in phases:
            return
        if timing_mode:
            # local-cost stand-in: write own shard into the replica
            nc.sync.dma_start(out=outs[0][0:NLOC, :], in_=ins[0])
            return
        nc.gpsimd.collective_compute(
            "AllGather", mybir.AluOpType.bypass,
            replica_groups=[list(range(NC))], ins=ins, outs=outs,
        )

    blob_d = nc.dram_tensor("blob", [plan.blob_len], i16, kind="ExternalInput")
    out_d = nc.dram_tensor("outp", [NLOC, cfg.C], bf16, kind="ExternalOutput")

    def bview(name, n, dt, p):
        """[p, n/p] view of blob section `name` with dtype dt (n in dt elems)."""
        n16 = {f32: 2 * n, i8: n // 2}.get(dt, n)
        v = blob_d[O[name] : O[name] + n16]
        if dt != i16:
            v = v.bitcast(dt)
        return v.rearrange("(p f) -> p f", p=p)

    ag_in = nc.dram_tensor("ag_in", [NLOC, H], f32)
    rep = [
        nc.dram_tensor(f"rep{j}", [NREP, H], f32, addr_space="Shared")
        for j in range(2)
    ]

    R = [float(v) for v in rcoef]
    off_l = np.concatenate([[0], np.cumsum(plan.cbs[:, 0])]).astype(int)
    off_h = np.concatenate([[0], np.cumsum(plan.cbs[:, 1])]).astype(int)

    with tile.TileContext(nc) as tc:
        with (
            tc.tile_pool(name="const", bufs=1) as constp,
            tc.tile_pool(name="xin", bufs=4) as xinp,
            tc.tile_pool(name="xun", bufs=3) as xunp,
            tc.tile_pool(name="msgl", bufs=2) as msglp,
            tc.tile_pool(name="msgh", bufs=2) as msghp,
            tc.tile_pool(name="msglb", bufs=2) as msglbp,
            tc.tile_pool(name="msghb", bufs=2) as msghbp,
            tc.tile_pool(name="wones", bufs=4) as wp,
            tc.tile_pool(name="upd", bufs=4) as updp,
            tc.tile_pool(name="head", bufs=3) as headp,
            tc.tile_pool(name="psum", bufs=4, space="PSUM") as psump,
            tc.tile_pool(name="psumh", bufs=2, space="PSUM") as psumhp,
        ):
            # ---------- resident tiles
            iota16_t = constp.tile([128, 128], i16)
            iota_t = constp.tile([128, 128], bf16)
            iotaf_t = constp.tile([128, 128], f32)
            pid16_t = constp.tile([128, 1], i16)
            pidf_t = constp.tile([128, 1], f32)
            ident_t = constp.tile([128, 128], f32)
            srct_t = constp.tile([128, nch], f32)
            wt_t = constp.tile([128, nch], f32)
            srct8_t = constp.tile([128, nch], i8)
            wtb_t = constp.tile([128, nch], bf16)
            idxl_t = constp.tile([128, nil], i16)
            idxh_t = constp.tile([128, nih], i16)
            m2w_t = constp.tile([cfg.H + 1, cfg.C], f32)
            alpha_t = constp.tile([128, BLOCKS], bf16)
            a05_t = constp.tile([128, BLOCKS], f32)
            x0_t = constp.tile([128, BLOCKS, H], f32)
            y_t = constp.tile([128, BLOCKS, H], f32)
            out_sb = constp.tile([128, BLOCKS, cfg.C], bf16)

            for t, name, n, dt, p in [
                (srct8_t, "srct", 128 * nch, i8, 128),
                (wtb_t, "wt", 128 * nch, bf16, 128),
                (m2w_t, "m2w", (cfg.H + 1) * cfg.C, f32, cfg.H + 1),
                (alpha_t, "alpha", 128 * BLOCKS, bf16, 128),
            ]:
                nc.sync.dma_start(out=t[:], in_=bview(name, n, dt, p))
            # iota / partition-id / identity built on device
            nc.gpsimd.iota(iota16_t[:], [[1, 128]], channel_multiplier=0)
            nc.gpsimd.iota(pid16_t[:], [[0, 1]], channel_multiplier=1)
            nc.vector.tensor_copy(iota_t[:], iota16_t[:])
            nc.vector.tensor_copy(iotaf_t[:], iota16_t[:])
            nc.vector.tensor_copy(pidf_t[:], pid16_t[:])
            nc.vector.tensor_scalar(ident_t[:], iotaf_t[:], pidf_t[:], None,
                                    Alu.is_equal)
            # gather indices: load the 16-row band 8x from DRAM to fill 128
            for t, name, w in [(idxl_t, "idxl", nil), (idxh_t, "idxh", nih)]:
                for k in range(8):
                    nc.sync.dma_start(out=t[16 * k : 16 * (k + 1), :],
                                      in_=bview(name, 16 * w, i16, 16))
            # m1w: KP > 128 partitions -> load as KCH separate [128, H] tiles
            m1w_ts = []
            for kc in range(KCH):
                mt = constp.tile([128, H], bf16, tag=f"m1w{kc}")
                nc.sync.dma_start(
                    out=mt[:],
                    in_=blob_d[O["m1w"] + kc * 128 * H : O["m1w"] + (kc + 1) * 128 * H]
                    .bitcast(bf16).rearrange("(p f) -> p f", p=128))
                m1w_ts.append(mt)

            nc.vector.tensor_copy(srct_t[:], srct8_t[:])
            nc.vector.tensor_copy(wt_t[:], wtb_t[:])
            nc.scalar.activation(a05_t[:], alpha_t[:],
                                 mybir.ActivationFunctionType.Sigmoid)
            nc.vector.tensor_scalar_mul(a05_t[:], a05_t[:], 0.5)

            # ---------- encoder: x0 = x @ m1_w + b ; y = r16 * x0
            inv_s = 1.0 / plan.xscale
            M = 0x3FF
            for b in range(BLOCKS if "e" in phases else 0):
                pe = psump.tile([128, H], f32, tag="acc")
                xq = xinp.tile([128, KCH, 5, 16], i16)
                nc.sync.dma_start(
                    out=xq[:],
                    in_=blob_d[O["xtb10"] + b * KP * 80 : O["xtb10"] + (b + 1) * KP * 80]
                    .rearrange("(kc p w g) -> p kc w g", p=128, w=5, g=16))
                xt = xunp.tile([128, KCH, 128], bf16, tag="xt")
                w0 = xq[:, :, 0, :]
                w1 = xq[:, :, 1, :]
                w2 = xq[:, :, 2, :]
                w3 = xq[:, :, 3, :]
                w4 = xq[:, :, 4, :]
                vt = [xunp.tile([128, KCH, 16], i16, tag=f"v{s}", name=f"v{s}")
                      for s in range(8)]
                va = xunp.tile([128, KCH, 16], i16, tag="va")
                ts = nc.vector.tensor_scalar
                # v0 = (w0 >> 6) & 0x3FF
                ts(vt[0][:], w0, 6, M, Alu.logical_shift_right, Alu.bitwise_and)
                # v1 = ((w0 & 0x3F) << 4) | ((w1 >> 12) & 0xF)
                ts(vt[1][:], w0, 0x3F, 4, Alu.bitwise_and, Alu.logical_shift_left)
                ts(va[:], w1, 12, 0xF, Alu.logical_shift_right, Alu.bitwise_and)
                nc.vector.tensor_tensor(vt[1][:], vt[1][:], va[:], Alu.bitwise_or)
                # v2 = (w1 >> 2) & 0x3FF
                ts(vt[2][:], w1, 2, M, Alu.logical_shift_right, Alu.bitwise_and)
                # v3 = ((w1 & 0x3) << 8) | ((w2 >> 8) & 0xFF)
                ts(vt[3][:], w1, 0x3, 8, Alu.bitwise_and, Alu.logical_shift_left)
                ts(va[:], w2, 8, 0xFF, Alu.logical_shift_right, Alu.bitwise_and)
                nc.vector.tensor_tensor(vt[3][:], vt[3][:], va[:], Alu.bitwise_or)
                # v4 = ((w2 & 0xFF) << 2) | ((w3 >> 14) & 0x3)
                ts(vt[4][:], w2, 0xFF, 2, Alu.bitwise_and, Alu.logical_shift_left)
                ts(va[:], w3, 14, 0x3, Alu.logical_shift_right, Alu.bitwise_and)
                nc.vector.tensor_tensor(vt[4][:], vt[4][:], va[:], Alu.bitwise_or)
                # v5 = (w3 >> 4) & 0x3FF
                ts(vt[5][:], w3, 4, M, Alu.logical_shift_right, Alu.bitwise_and)
                # v6 = ((w3 & 0xF) << 6) | ((w4 >> 10) & 0x3F)
                ts(vt[6][:], w3, 0xF, 6, Alu.bitwise_and, Alu.logical_shift_left)
                ts(va[:], w4, 10, 0x3F, Alu.logical_shift_right, Alu.bitwise_and)
                nc.vector.tensor_tensor(vt[6][:], vt[6][:], va[:], Alu.bitwise_or)
                # v7 = w4 & 0x3FF
                ts(vt[7][:], w4, M, 0, Alu.bitwise_and, Alu.logical_shift_right)
                for s in range(8):
                    nc.scalar.activation(
                        xt[:, :, s::8], vt[s][:],
                        mybir.ActivationFunctionType.Copy,
                        scale=inv_s, bias=-512.0 * inv_s)
                for kc in range(KCH):
                    nc.tensor.matmul(pe[:], xt[:, kc, :], m1w_ts[kc][:],
                                     start=(kc == 0), stop=(kc == KCH - 1))
                nc.scalar.activation(x0_t[:, b, :], pe[:],
                                     mybir.ActivationFunctionType.Copy)
                nc.vector.tensor_scalar_mul(y_t[:, b, :], pe[:], R[cfg.NITER])

            # publish y -> replica 0
            agv = ag_in[:].rearrange("(b p) f -> p b f", p=128)
            nc.sync.dma_start(out=agv, in_=y_t[:])
            allgather([ag_in[:]], [rep[0][:]])

            # ---------- Horner iterations (flat; collectives cannot sit in
            # a For_i loop on this toolchain)
            nidx_reg = nc.gpsimd.to_reg(GCH * 128)

            def horner_iter(src_rep, dst_rep, rk_imm, last):
                # gathers for both streams; convert messages to bf16
                msg_tiles = {0: [], 1: []}
                for s, (pool, poolb, idx_t, ngrp, base) in enumerate([
                    (msglp, msglbp, idxl_t, plan.ngrp_l, 0),
                    (msghp, msghbp, idxh_t, plan.ngrp_h, HALF),
                ]):
                    view = src_rep[base : base + HALF, :]
                    for g in range(ngrp):
                        mt = pool.tile([128, GCH, H], f32, tag=f"msg{s}")
                        for _ in range(RG):
                            nc.gpsimd.dma_gather(
                                mt[:], view,
                                idx_t[:, g * GCH * 8 : (g + 1) * GCH * 8],
                                GCH * 128, nidx_reg, H, elem_step=H,
                                single_packet=False)
                        mtb = poolb.tile([128, GCH, H], bf16, tag=f"msgb{s}")
                        nc.vector.tensor_copy(mtb[:], mt[:])
                        msg_tiles[s].append(mtb)

                for b in range(BLOCKS):
                    ps = psump.tile([128, H], f32, tag="acc")
                    ncl = int(plan.cbs[b, 0])
                    nchh = int(plan.cbs[b, 1])
                    tot = ncl + nchh
                    for rm in range(RM):
                        ci = 0
                        for s, n_s, offc in ((0, ncl, off_l), (1, nchh, off_h)):
                            for j in range(n_s):
                                col = offc[b] + j            # stream-chunk idx
                                tabcol = col + (plan.nch_l if s else 0)
                                wt_tile = wp.tile([128, 128], bf16, tag="W")
                                nc.vector.tensor_scalar(
                                    wt_tile[:], iota_t[:],
                                    srct_t[:, tabcol : tabcol + 1],
                                    wt_t[:, tabcol : tabcol + 1],
                                    Alu.is_equal, Alu.mult)
                                mtb = msg_tiles[s][col // GCH]
                                nc.tensor.matmul(
                                    ps[:], wt_tile[:], mtb[:, col % GCH, :],
                                    start=(ci == 0 and rm == 0),
                                    stop=(ci == tot - 1 and rm == RM - 1),
                                    skip_group_check=True)
                                ci += 1
                    # y' = a05*(az - y) + r_k*x0  == a05*az - (a05*y - r_k*x0)
                    for ru in range(RU):
                        x0s = updp.tile([128, H], f32, tag="x0s")
                        nc.scalar.activation(
                            x0s[:], x0_t[:, b, :],
                            mybir.ActivationFunctionType.Copy, scale=rk_imm)
                        tt = updp.tile([128, H], f32, tag="tt")
                        nc.vector.scalar_tensor_tensor(
                            tt[:], y_t[:, b, :], a05_t[:, b : b + 1], x0s[:],
                            Alu.mult, Alu.subtract)
                        nc.vector.scalar_tensor_tensor(
                            y_t[:, b, :], ps[:], a05_t[:, b : b + 1], tt[:],
                            Alu.mult, Alu.subtract)

                if not last:
                    # publish y for the next iteration
                    nc.sync.dma_start(out=agv, in_=y_t[:])
                    allgather([ag_in[:]], [dst_rep[:]])

            for i in range(cfg.NITER if "h" in phases else 0):
                horner_iter(rep[i % 2], rep[(i + 1) % 2],
                            R[cfg.NITER - 1 - i], i == cfg.NITER - 1)

            # ---------- head: out = relu(y) @ m2_w + b  (all f32)
            for b in range(BLOCKS if "d" in phases else 0):
                rt = headp.tile([128, H], f32, tag="relu")
                nc.scalar.activation(rt[:], y_t[:, b, :],
                                     mybir.ActivationFunctionType.Relu)
                pt = psumhp.tile([H, 128], f32, tag="tp")
                nc.tensor.transpose(pt[:], rt[:], ident_t[:])
                rta = headp.tile([H + 1, 128], f32, tag="rta")
                nc.vector.memset(rta[H : H + 1, :], 1.0)
                nc.vector.tensor_copy(rta[0:H, :], pt[:])
                po = psumhp.tile([128, cfg.C], f32, tag="po")
                nc.tensor.matmul(po[:], rta[:], m2w_t[:])
                nc.vector.tensor_copy(out_sb[:, b, :], po[:])

            outv = out_d[:].rearrange("(b p) f -> p b f", p=128)
            nc.sync.dma_start(out=outv, in_=out_sb[:])

    nc.finalize()

    # The program is immutable from here on; memoize its JSON serialization
    # on this instance (the jax lowering re-serializes the BIR every call).
    orig_to_json_bytes = nc.to_json_bytes
    cache = []

    def _memo_to_json_bytes():
        if not cache:
            cache.append(orig_to_json_bytes())
        return cache[0]

    nc.to_json_bytes = _memo_to_json_bytes
    return nc


# ------------------------------------------------------------ entry point
def kernel(**inputs) -> np.ndarray:
    cfg = Cfg()
    rcoef = horner_coeffs(cfg)
    plan = build_plan(cfg, inputs)
    nc = build_program(cfg, plan, rcoef)

    from concourse.bass_utils import run_bass_kernel_spmd

    res = run_bass_kernel_spmd(nc, plan.in_maps, list(range(cfg.NCORES)))
    out = np.concatenate(
        [res.results[c]["outp"][: cfg.NSH] for c in range(cfg.NCORES)], axis=0
    )
    return out.astype(np.float32)


# revision 40
# speedup vs baseline: 1.0232x; 1.0232x over previous
"""Trainium2 Bass kernel for CGNN message-passing ODE (nn_CGNN_51333449121989).

Math: the reference integrates the affine ODE z' = diag(sigmoid(alpha))*0.5*(A z - z) + x0
with RK4 (4 steps, dt=0.25) from z0 = x0, where x0 = [x @ m1_w + m1_b, zeros].
Since each RK4 step is the affine map z <- P(M) z + Q(M) x0 with
M = diag(a)*0.5*(A - I), the final state is an exact degree-16 polynomial
R(M) x0, evaluated here by 16 Horner iterations:
    y <- a05 * (A y - y) + r_k * x0      (a05 = 0.5*sigmoid(alpha))
Feature columns H..2H-1 of the state are identically zero (columns evolve
independently and start/force at zero), so the working state is [N, H].

Distribution: 1D node partition over 8 cores (6250 rows each, padded to
6272 = 49*128).  Each core owns the edges whose src falls in its row range.
Per iteration each core:
  - dma_gather's y[dst] rows (256B each) from a full HBM replica of y
    (int16 gather indices => the replica is split in two halves; edges are
    routed into a "low" and a "high" gather stream),
  - segment-sums messages into psum per 128-row block with PE matmuls:
    psum[128 rows, H] += onehot(src_local)^T @ msg, where the per-chunk
    onehot-with-weights matrix W[e, r] = (r == src_local[e]) * w_e is built
    on the vector engine from an iota tile via a dual-op tensor_scalar,
  - applies the Horner update, publishes its shard and AllGathers the next
    replica.
The 16 iterations are flat python-unrolled: collectives cannot execute
inside a For_i hardware loop on this toolchain, and dma_scatter_add loses
colliding updates, so the onehot-matmul segment-sum stays.

Transfer layer: the per-call wall time is dominated by host->device upload
through the PJRT tunnel (~40MB/s for incompressible bytes, large per-array
fixed cost).  Countermeasures:
  - ONE flat int16 blob per core holds every input (9-bit fixed-point x
    unpacked on device via integer shift/mask DVE ops; bf16 tables; int8
    src rows and edge weights; int16 gather indices uploaded once at 16 rows and replicated
    to 128 on device; iota/identity built on device), output returned as
    bf16;
  - jax persistent compilation cache so warm calls skip the walrus backend
    compile, plus a per-instance memo of the BIR JSON serialization that
    the jax lowering otherwise recomputes every call.
Host-side work is limited to sharding/packing: edge bucketing + padding to a
core-uniform chunk structure, gather-index wrapping, and input layout.
"""

import os
import sys

sys.path.insert(0, "/opt/trn_rl_repo")

from dataclasses import dataclass

import numpy as np
import ml_dtypes


def _setup_jax_compilation_cache():
    """Persistent XLA compilation cache: warm run_bass_kernel_spmd calls hit
    the cache instead of re-running the (~2s) walrus backend compile."""
    try:
        import jax

        jax.config.update("jax_compilation_cache_dir", "/tmp/jax_comp_cache")
        jax.config.update("jax_persistent_cache_min_compile_time_secs", 0)
        jax.config.update("jax_persistent_cache_min_entry_size_bytes", -1)
    except Exception:
        pass


_setup_jax_compilation_cache()


# ---------------------------------------------------------------- constants
@dataclass(frozen=True)
class Cfg:
    N: int = 50000          # nodes
    E: int = 600000         # edges
    F: int = 500            # input features
    H: int = 64             # hidden (ODE state width)
    C: int = 40             # classes
    NCORES: int = 8
    NITER: int = 16         # Horner iterations (degree-16 polynomial, exact)
    DT: float = 0.25        # T / STEPS from the reference
    GCH: int = 48           # gather-group size in chunks (48*128 idx per call)

    @property
    def NSH(self):          # true rows per core
        return self.N // self.NCORES

    @property
    def BLOCKS(self):       # 128-row blocks per core
        return (self.NSH + 127) // 128

    @property
    def NLOC(self):         # padded rows per core
        return self.BLOCKS * 128

    @property
    def NREP(self):         # replica rows
        return self.NCORES * self.NLOC

    @property
    def HALF(self):         # low/high split of replica rows (int16 gather idx)
        return self.NREP // 2

    @property
    def KP(self):           # encoder contraction dim padded (F + bias row)
        return ((self.F + 1 + 127) // 128) * 128


def horner_coeffs(cfg: Cfg) -> np.ndarray:
    """Coefficients r_0..r_16 of the exact RK4 polynomial R(M)."""
    dt = cfg.DT
    deg = max(cfg.NITER, 16)
    P = np.zeros(deg + 1)
    Q = np.zeros(deg + 1)
    P[0] = 1.0
    fact = 1.0
    for j in range(1, 5):
        fact *= j
        P[j] = dt**j / fact
        Q[j - 1] = dt**j / fact

    def pmul(a, b):
        out = np.zeros(2 * deg + 1)
        for i in range(deg + 1):
            if a[i]:
                out[i : i + deg + 1] += a[i] * b
        return out[: deg + 1]

    P2 = pmul(P, P)
    P3 = pmul(P2, P)
    P4 = pmul(P3, P)
    S = P3 + P2 + P
    S[0] += 1.0
    R = P4 + pmul(S, Q)
    return R


# ------------------------------------------------------------ tile patch
def _patch_tile_drain():
    """This toolchain's walrus rejects instructions with several sem waits;
    split TileContext's exit-drain waits across single-wait nops."""
    import concourse.tile as tile
    from concourse.vector_clock import ScopedClock
    from bass_rust import VectorClock

    if getattr(tile.TileContext, "_drain_patched", False):
        return

    def _drain_and_barrier(self, tick_clock, wait_clock):
        gc = tick_clock.global_clock
        scoped = ScopedClock({None: gc})
        for scope, vc in scoped.items():
            procs = [i for i in range(len(vc)) if vc[i] > 0]
            for p in procs:
                pvc = VectorClock()
                pvc.require_at_least(p, vc[p])
                nop = self.nc.sync.nop(nofuse=True, hint="drain_split")
                wait_clock.add_sem_waits(nop.ins, ScopedClock({scope: pvc}))
        self.nc.sync.drain()
        self.nc.all_engine_barrier()
        assert self.sems is not None
        popped = self.nc._tile_sem_poison_stack.pop()
        assert popped is self._sem_poison
        self.nc.clear_and_free_semaphores(list(self.sems.allocated().values()))
        self.nc.all_engine_barrier()

    tile.TileContext._drain_and_barrier = _drain_and_barrier
    tile.TileContext._drain_patched = True


# ------------------------------------------------------------ host prep
def _align(v, a=128):
    return (v + a - 1) // a * a


@dataclass
class Plan:
    # uniform chunk structure
    nch_l: int
    nch_h: int
    cbs: np.ndarray           # [BLOCKS, 2] chunks per (block, stream)
    ngrp_l: int
    ngrp_h: int
    # 10-bit x quantization scale (global)
    xscale: float
    # blob section offsets (int16 elements), core-uniform
    off: dict
    blob_len: int
    # per-core packed tensors
    in_maps: list


def build_plan(cfg: Cfg, inputs: dict) -> Plan:
    x = np.asarray(inputs["x"], np.float32)
    ew = np.asarray(inputs["edge_w"], np.float32)
    src = np.asarray(inputs["edge_src"], np.int64)
    dst = np.asarray(inputs["edge_dst"], np.int64)
    m1w = np.asarray(inputs["m1_w"], np.float32)
    m1b = np.asarray(inputs["m1_b"], np.float32)
    alpha = np.asarray(inputs["alpha_train"], np.float32)
    m2w = np.asarray(inputs["m2_w"], np.float32)
    m2b = np.asarray(inputs["m2_b"], np.float32)
    bf16 = ml_dtypes.bfloat16

    NC, NSH, NLOC, BLOCKS = cfg.NCORES, cfg.NSH, cfg.NLOC, cfg.BLOCKS
    HALF, GCH, KP = cfg.HALF, cfg.GCH, cfg.KP

    owner = src // NSH
    owner = np.minimum(owner, NC - 1)
    src_loc = src - owner * NSH
    downer = dst // NSH
    downer = np.minimum(downer, NC - 1)
    dpos = downer * NLOC + (dst - downer * NSH)   # replica row of dst
    stream = (dpos >= HALF).astype(np.int64)      # 0 = low, 1 = high
    block = src_loc // 128
    srow = src_loc % 128                          # row within block

    # ---- per-(core, block, stream) edge buckets
    counts = np.zeros((NC, BLOCKS, 2), np.int64)
    np.add.at(counts, (owner, block, stream), 1)
    cbs = np.ceil(counts.max(axis=0) / 128).astype(np.int64)   # [BLOCKS, 2]
    cbs[:, 0] = np.maximum(cbs[:, 0], 1)    # every block needs >=1 chunk
    nch_l = int(cbs[:, 0].sum())
    nch_h = int(cbs[:, 1].sum())
    nch = nch_l + nch_h
    ngrp_l = (nch_l + GCH - 1) // GCH
    ngrp_h = (nch_h + GCH - 1) // GCH
    nil = ngrp_l * GCH * 8        # idx columns per 16-row band, low
    nih = ngrp_h * GCH * 8

    # chunk column offsets per (block, stream); stream H columns offset by nch_l
    off_l = np.concatenate([[0], np.cumsum(cbs[:, 0])])
    off_h = np.concatenate([[0], np.cumsum(cbs[:, 1])])

    # ---- 9-bit fixed point for x: q = round(x*s) in [-255, 255]
    xscale = 255.0 / max(float(np.abs(x).max()), 1e-6)

    # ---- blob layout (int16 elements, 128-aligned sections, core-uniform)
    off = {}
    pos = 0
    def sect(name, n):
        nonlocal pos
        off[name] = pos
        pos += _align(n)
    sect("xtb9", BLOCKS * KP * 72)          # packed 9-bit [BLOCKS, KP, 9, 8]
    sect("m1w", KP * cfg.H)                 # bf16 [KP, H]
    sect("m2w", (cfg.H + 1) * cfg.C * 2)    # f32  [H+1, C]
    sect("alpha", 128 * BLOCKS)             # bf16 [128, BLOCKS]
    sect("srct", 64 * nch)                  # int8 [128, nch]
    sect("wt", 64 * nch)                    # int8 [128, nch], (v+128)/255
    sect("idxl", 16 * nil)                  # int16 [16, nil]
    sect("idxh", 16 * nih)                  # int16 [16, nih]
    blob_len = _align(pos, 1024)

    m1w_aug = np.zeros((KP, cfg.H), np.float32)
    m1w_aug[: cfg.F] = m1w
    m1w_aug[cfg.F] = m1b
    m1w_b = m1w_aug.astype(bf16)
    m2w_aug = np.zeros((cfg.H + 1, cfg.C), np.float32)
    m2w_aug[: cfg.H] = m2w
    m2w_aug[cfg.H] = m2b

    def put(blob, name, arr):
        a16 = np.ascontiguousarray(arr).reshape(-1).view(np.int16)
        blob[off[name] : off[name] + a16.size] = a16

    def pack9(vals):
        """vals [R, 128] signed ints in [-256, 255] -> [R, 9, 8] uint16."""
        q = (vals.astype(np.int32) + 256).astype(np.uint32).reshape(-1, 8, 16)
        qq = [q[:, :, i] for i in range(16)]
        w = [
            (qq[0] << 7) | (qq[1] >> 2),
            ((qq[1] & 0x3) << 14) | (qq[2] << 5) | (qq[3] >> 4),
            ((qq[3] & 0xF) << 12) | (qq[4] << 3) | (qq[5] >> 6),
            ((qq[5] & 0x3F) << 10) | (qq[6] << 1) | (qq[7] >> 8),
            ((qq[7] & 0xFF) << 8) | (qq[8] >> 1),
            ((qq[8] & 0x1) << 15) | (qq[9] << 6) | (qq[10] >> 3),
            ((qq[10] & 0x7) << 13) | (qq[11] << 4) | (qq[12] >> 5),
            ((qq[12] & 0x1F) << 11) | (qq[13] << 2) | (qq[14] >> 7),
            ((qq[14] & 0x7F) << 9) | qq[15],
        ]
        return np.stack(w, axis=1).astype(np.uint16)   # [R, 9, 8]

    # sort edges per core by (stream, block, dst) for gather locality
    in_maps = []
    for c in range(NC):
        sel = owner == c
        eb, es, er, ed, ewc = (
            block[sel], stream[sel], srow[sel], dpos[sel], ew[sel])

        src_tab = np.zeros((128, nch), np.int8)
        w_tab = np.full((128, nch), -128, np.int8)   # -128 -> w = 0.0
        idx_l = np.zeros(nch_l * 128, np.int64)
        idx_h = np.zeros(nch_h * 128, np.int64)

        for s, (idx_arr, offc, base_col) in enumerate(
            [(idx_l, off_l, 0), (idx_h, off_h, nch_l)]
        ):
            m = es == s
            b_s, r_s, d_s, w_s = eb[m], er[m], ed[m], ewc[m]
            order = np.lexsort((d_s, b_s))
            b_s, r_s, d_s, w_s = b_s[order], r_s[order], d_s[order], w_s[order]
            # place edges of block b into its chunk range [offc[b], offc[b+1])
            starts = np.searchsorted(b_s, np.arange(BLOCKS))
            ends = np.searchsorted(b_s, np.arange(BLOCKS), side="right")
            for b in range(BLOCKS):
                n_edges = ends[b] - starts[b]
                pos0 = offc[b] * 128
                sl = slice(starts[b], ends[b])
                idx_arr[pos0 : pos0 + n_edges] = d_s[sl] - s * HALF
                cols = np.arange(n_edges) // 128 + offc[b]
                rows = np.arange(n_edges) % 128
                src_tab[rows, base_col + cols] = r_s[sl].astype(np.int8)
                w_tab[rows, base_col + cols] = (
                    np.round(w_s[sl] * 255.0).astype(np.int32) - 128
                ).astype(np.int8)
                # padding edges keep w=0 / idx=0 / src_row=0

        def wrap_idx(idx_arr, ngrp):
            n = ngrp * GCH * 128
            full = np.zeros(n, np.int64)
            full[: len(idx_arr)] = idx_arr
            return full.reshape(-1, 16).T.astype(np.int16)    # [16, n/16]

        # encoder inputs: block-tiled x^T with bias row, 10-bit packed
        rows = slice(c * NSH, (c + 1) * NSH)
        xsh = np.zeros((NLOC, cfg.F), np.float32)
        xsh[:NSH] = x[rows]
        xtb_q = np.zeros((BLOCKS, KP, 128), np.int32)
        for b in range(BLOCKS):
            xtb_q[b, : cfg.F, :] = np.round(
                xsh[b * 128 : (b + 1) * 128].T * xscale).astype(np.int32)
            xtb_q[b, cfg.F, :] = int(round(xscale))     # bias row ~ 1.0
        xtb_p = pack9(xtb_q.reshape(-1, 128)).reshape(BLOCKS, KP, 9, 8)

        al = np.zeros(NLOC, np.float32)
        al[:NSH] = alpha[rows]
        alpha_b = al.reshape(BLOCKS, 128).T.astype(bf16)    # [128, BLOCKS]

        blob = np.zeros(blob_len, np.int16)
        put(blob, "xtb9", xtb_p)
        put(blob, "m1w", m1w_b)
        put(blob, "m2w", m2w_aug)
        put(blob, "alpha", alpha_b)
        put(blob, "srct", src_tab)
        put(blob, "wt", w_tab)
        put(blob, "idxl", wrap_idx(idx_l, ngrp_l))
        put(blob, "idxh", wrap_idx(idx_h, ngrp_h))
        in_maps.append(dict(blob=blob))

    return Plan(nch_l, nch_h, np.asarray(cbs), ngrp_l, ngrp_h,
                xscale, off, blob_len, in_maps)


# ------------------------------------------------------------ device program
def build_program(cfg: Cfg, plan: Plan, rcoef: np.ndarray,
                  timing_mode: bool = False, phases: str = "ehda",
                  reps=(1, 1, 1)):
    RG, RM, RU = reps   # timing: repeat gathers / matmuls / updates
    """timing_mode: single-core variant for TimelineSim (collectives replaced
    by a local DMA of the same local traffic)."""
    import concourse.bacc as bacc
    import concourse.mybir as mybir
    import concourse.tile as tile

    _patch_tile_drain()

    NC, H, BLOCKS, NLOC, NREP, HALF = (
        cfg.NCORES, cfg.H, cfg.BLOCKS, cfg.NLOC, cfg.NREP, cfg.HALF)
    GCH, KP = cfg.GCH, cfg.KP
    KCH = KP // 128
    f32 = mybir.dt.float32
    bf16 = mybir.dt.bfloat16
    i16 = mybir.dt.int16
    i8 = mybir.dt.int8
    Alu = mybir.AluOpType
    nch = plan.nch_l + plan.nch_h
    nil = plan.ngrp_l * GCH * 8
    nih = plan.ngrp_h * GCH * 8
    O = plan.off

    nc = bacc.Bacc("TRN2", target_bir_lowering=False, debug=False,
                   num_devices=1 if timing_mode else NC)

    def allgather(ins, outs):
        if "a" not in phases:
            return
        if timing_mode:
            # local-cost stand-in: write own shard into the replica
            nc.sync.dma_start(out=outs[0][0:NLOC, :], in_=ins[0])
            return
        nc.gpsimd.collective_compute(
            "AllGather", mybir.AluOpType.bypass,
            replica_groups=[list(range(NC))], ins=ins, outs=outs,
        )

    blob_d = nc.dram_tensor("blob", [plan.blob_len], i16, kind="ExternalInput")
    out_d = nc.dram_tensor("outp", [NLOC, cfg.C], bf16, kind="ExternalOutput")

    def bview(name, n, dt, p):
        """[p, n/p] view of blob section `name` with dtype dt (n in dt elems)."""
        n16 = {f32: 2 * n, i8: n // 2}.get(dt, n)
        v = blob_d[O[name] : O[name] + n16]
        if dt != i16:
            v = v.bitcast(dt)
        return v.rearrange("(p f) -> p f", p=p)

    ag_in = nc.dram_tensor("ag_in", [NLOC, H], f32)
    rep = [
        nc.dram_tensor(f"rep{j}", [NREP, H], f32, addr_space="Shared")
        for j in range(2)
    ]

    R = [float(v) for v in rcoef]
    off_l = np.concatenate([[0], np.cumsum(plan.cbs[:, 0])]).astype(int)
    off_h = np.concatenate([[0], np.cumsum(plan.cbs[:, 1])]).astype(int)

    with tile.TileContext(nc) as tc:
        with (
            tc.tile_pool(name="const", bufs=1) as constp,
            tc.tile_pool(name="xin", bufs=4) as xinp,
            tc.tile_pool(name="xun", bufs=3) as xunp,
            tc.tile_pool(name="msgl", bufs=2) as msglp,
            tc.tile_pool(name="msgh", bufs=2) as msghp,
            tc.tile_pool(name="msglb", bufs=2) as msglbp,
            tc.tile_pool(name="msghb", bufs=2) as msghbp,
            tc.tile_pool(name="wones", bufs=4) as wp,
            tc.tile_pool(name="upd", bufs=4) as updp,
            tc.tile_pool(name="head", bufs=3) as headp,
            tc.tile_pool(name="psum", bufs=4, space="PSUM") as psump,
            tc.tile_pool(name="psumh", bufs=2, space="PSUM") as psumhp,
        ):
            # ---------- resident tiles
            iota16_t = constp.tile([128, 128], i16)
            iota_t = constp.tile([128, 128], bf16)
            iotaf_t = constp.tile([128, 128], f32)
            pid16_t = constp.tile([128, 1], i16)
            pidf_t = constp.tile([128, 1], f32)
            ident_t = constp.tile([128, 128], f32)
            srct_t = constp.tile([128, nch], f32)
            wt_t = constp.tile([128, nch], f32)
            srct8_t = constp.tile([128, nch], i8)
            wt8_t = constp.tile([128, nch], i8)
            idxl_t = constp.tile([128, nil], i16)
            idxh_t = constp.tile([128, nih], i16)
            m2w_t = constp.tile([cfg.H + 1, cfg.C], f32)
            alpha_t = constp.tile([128, BLOCKS], bf16)
            a05_t = constp.tile([128, BLOCKS], f32)
            x0_t = constp.tile([128, BLOCKS, H], f32)
            y_t = constp.tile([128, BLOCKS, H], f32)
            out_sb = constp.tile([128, BLOCKS, cfg.C], bf16)

            for t, name, n, dt, p in [
                (srct8_t, "srct", 128 * nch, i8, 128),
                (wt8_t, "wt", 128 * nch, i8, 128),
                (m2w_t, "m2w", (cfg.H + 1) * cfg.C, f32, cfg.H + 1),
                (alpha_t, "alpha", 128 * BLOCKS, bf16, 128),
            ]:
                nc.sync.dma_start(out=t[:], in_=bview(name, n, dt, p))
            # iota / partition-id / identity built on device
            nc.gpsimd.iota(iota16_t[:], [[1, 128]], channel_multiplier=0)
            nc.gpsimd.iota(pid16_t[:], [[0, 1]], channel_multiplier=1)
            nc.vector.tensor_copy(iota_t[:], iota16_t[:])
            nc.vector.tensor_copy(iotaf_t[:], iota16_t[:])
            nc.vector.tensor_copy(pidf_t[:], pid16_t[:])
            nc.vector.tensor_scalar(ident_t[:], iotaf_t[:], pidf_t[:], None,
                                    Alu.is_equal)
            # gather indices: load the 16-row band 8x from DRAM to fill 128
            for t, name, w in [(idxl_t, "idxl", nil), (idxh_t, "idxh", nih)]:
                for k in range(8):
                    nc.sync.dma_start(out=t[16 * k : 16 * (k + 1), :],
                                      in_=bview(name, 16 * w, i16, 16))
            # m1w: KP > 128 partitions -> load as KCH separate [128, H] tiles
            m1w_ts = []
            for kc in range(KCH):
                mt = constp.tile([128, H], bf16, tag=f"m1w{kc}")
                nc.sync.dma_start(
                    out=mt[:],
                    in_=blob_d[O["m1w"] + kc * 128 * H : O["m1w"] + (kc + 1) * 128 * H]
                    .bitcast(bf16).rearrange("(p f) -> p f", p=128))
                m1w_ts.append(mt)

            nc.vector.tensor_copy(srct_t[:], srct8_t[:])
            # w = (v + 128) / 255
            nc.vector.tensor_scalar(wt_t[:], wt8_t[:], 128.0, 1.0 / 255.0,
                                    Alu.add, Alu.mult)
            nc.scalar.activation(a05_t[:], alpha_t[:],
                                 mybir.ActivationFunctionType.Sigmoid)
            nc.vector.tensor_scalar_mul(a05_t[:], a05_t[:], 0.5)

            # ---------- encoder: x0 = x @ m1_w + b ; y = r16 * x0
            inv_s = 1.0 / plan.xscale
            M9 = 0x1FF
            # stream spec: single-word  (wa, rshift, mask, None)
            #                -> v = (wa >> rshift) & mask
            #              dual-word    (wa, mask_a, lshift, (wb, rshift_b, mask_b))
            #                -> v = ((wa & mask_a) << lshift) | ((wb >> rshift_b) & mask_b)
            # (v15 = w8 & 0x1FF is special-cased below)
            UNPACK9 = [
                (0, 7, M9, None), (0, 0x7F, 2, (1, 14, 0x3)),
                (1, 5, M9, None), (1, 0x1F, 4, (2, 12, 0xF)),
                (2, 3, M9, None), (2, 0x7, 6, (3, 10, 0x3F)),
                (3, 1, M9, None), (3, 0x1, 8, (4, 8, 0xFF)),
                (4, 0xFF, 1, (5, 15, 0x1)),
                (5, 6, M9, None), (5, 0x3F, 3, (6, 13, 0x7)),
                (6, 4, M9, None), (6, 0xF, 5, (7, 11, 0x1F)),
                (7, 2, M9, None), (7, 0x3, 7, (8, 9, 0x7F)),
                (8, 0x1FF, 0, None),
            ]
            for b in range(BLOCKS if "e" in phases else 0):
                pe = psump.tile([128, H], f32, tag="acc")
                xq = xinp.tile([128, KCH, 9, 8], i16)
                nc.sync.dma_start(
                    out=xq[:],
                    in_=blob_d[O["xtb9"] + b * KP * 72 : O["xtb9"] + (b + 1) * KP * 72]
                    .rearrange("(kc p w g) -> p kc w g", p=128, w=9, g=8))
                xt = xunp.tile([128, KCH, 128], bf16, tag="xt")
                wv = [xq[:, :, j, :] for j in range(9)]
                va = xunp.tile([128, KCH, 8], i16, tag="va")
                vb = xunp.tile([128, KCH, 8], i16, tag="vb")
                ts = nc.vector.tensor_scalar
                for s, spec in enumerate(UNPACK9):
                    if spec[3] is None:
                        if s == 16 - 1:  # v15 = w8 & 0x1FF
                            ts(va[:], wv[8], M9, 0,
                               Alu.bitwise_and, Alu.logical_shift_right)
                        else:
                            wa, ra, ma, _ = spec
                            ts(va[:], wv[wa], ra, ma,
                               Alu.logical_shift_right, Alu.bitwise_and)
                        out_v = va
                    else:
                        wa, ma, la, (wb, rb, mb) = spec
                        ts(va[:], wv[wa], ma, la,
                           Alu.bitwise_and, Alu.logical_shift_left)
                        ts(vb[:], wv[wb], rb, mb,
                           Alu.logical_shift_right, Alu.bitwise_and)
                        nc.vector.tensor_tensor(va[:], va[:], vb[:],
                                                Alu.bitwise_or)
                        out_v = va
                    nc.scalar.activation(
                        xt[:, :, s::16], out_v[:],
                        mybir.ActivationFunctionType.Copy,
                        scale=inv_s, bias=-256.0 * inv_s)
                for kc in range(KCH):
                    nc.tensor.matmul(pe[:], xt[:, kc, :], m1w_ts[kc][:],
                                     start=(kc == 0), stop=(kc == KCH - 1))
                nc.scalar.activation(x0_t[:, b, :], pe[:],
                                     mybir.ActivationFunctionType.Copy)
                nc.vector.tensor_scalar_mul(y_t[:, b, :], pe[:], R[cfg.NITER])

            # publish y -> replica 0
            agv = ag_in[:].rearrange("(b p) f -> p b f", p=128)
            nc.sync.dma_start(out=agv, in_=y_t[:])
            allgather([ag_in[:]], [rep[0][:]])

            # ---------- Horner iterations (flat; collectives cannot sit in
            # a For_i loop on this toolchain)
            nidx_reg = nc.gpsimd.to_reg(GCH * 128)

            def horner_iter(src_rep, dst_rep, rk_imm, last):
                # gathers for both streams; convert messages to bf16
                msg_tiles = {0: [], 1: []}
                for s, (pool, poolb, idx_t, ngrp, base) in enumerate([
                    (msglp, msglbp, idxl_t, plan.ngrp_l, 0),
                    (msghp, msghbp, idxh_t, plan.ngrp_h, HALF),
                ]):
                    view = src_rep[base : base + HALF, :]
                    for g in range(ngrp):
                        mt = pool.tile([128, GCH, H], f32, tag=f"msg{s}")
                        for _ in range(RG):
                            nc.gpsimd.dma_gather(
                                mt[:], view,
                                idx_t[:, g * GCH * 8 : (g + 1) * GCH * 8],
                                GCH * 128, nidx_reg, H, elem_step=H,
                                single_packet=False)
                        mtb = poolb.tile([128, GCH, H], bf16, tag=f"msgb{s}")
                        nc.vector.tensor_copy(mtb[:], mt[:])
                        msg_tiles[s].append(mtb)

                for b in range(BLOCKS):
                    ps = psump.tile([128, H], f32, tag="acc")
                    ncl = int(plan.cbs[b, 0])
                    nchh = int(plan.cbs[b, 1])
                    tot = ncl + nchh
                    for rm in range(RM):
                        ci = 0
                        for s, n_s, offc in ((0, ncl, off_l), (1, nchh, off_h)):
                            for j in range(n_s):
                                col = offc[b] + j            # stream-chunk idx
                                tabcol = col + (plan.nch_l if s else 0)
                                wt_tile = wp.tile([128, 128], bf16, tag="W")
                                nc.vector.tensor_scalar(
                                    wt_tile[:], iota_t[:],
                                    srct_t[:, tabcol : tabcol + 1],
                                    wt_t[:, tabcol : tabcol + 1],
                                    Alu.is_equal, Alu.mult)
                                mtb = msg_tiles[s][col // GCH]
                                nc.tensor.matmul(
                                    ps[:], wt_tile[:], mtb[:, col % GCH, :],
                                    start=(ci == 0 and rm == 0),
                                    stop=(ci == tot - 1 and rm == RM - 1),
                                    skip_group_check=True)
                                ci += 1
                    # y' = a05*(az - y) + r_k*x0  == a05*az - (a05*y - r_k*x0)
                    for ru in range(RU):
                        x0s = updp.tile([128, H], f32, tag="x0s")
                        nc.scalar.activation(
                            x0s[:], x0_t[:, b, :],
                            mybir.ActivationFunctionType.Copy, scale=rk_imm)
                        tt = updp.tile([128, H], f32, tag="tt")
                        nc.vector.scalar_tensor_tensor(
                            tt[:], y_t[:, b, :], a05_t[:, b : b + 1], x0s[:],
                            Alu.mult, Alu.subtract)
                        nc.vector.scalar_tensor_tensor(
                            y_t[:, b, :], ps[:], a05_t[:, b : b + 1], tt[:],
                            Alu.mult, Alu.subtract)

                if not last:
                    # publish y for the next iteration
                    nc.sync.dma_start(out=agv, in_=y_t[:])
                    allgather([ag_in[:]], [dst_rep[:]])

            for i in range(cfg.NITER if "h" in phases else 0):
                horner_iter(rep[i % 2], rep[(i + 1) % 2],
                            R[cfg.NITER - 1 - i], i == cfg.NITER - 1)

            # ---------- head: out = relu(y) @ m2_w + b  (all f32)
            for b in range(BLOCKS if "d" in phases else 0):
                rt = headp.tile([128, H], f32, tag="relu")
                nc.scalar.activation(rt[:], y_t[:, b, :],
                                     mybir.ActivationFunctionType.Relu)
                pt = psumhp.tile([H, 128], f32, tag="tp")
                nc.tensor.transpose(pt[:], rt[:], ident_t[:])
                rta = headp.tile([H + 1, 128], f32, tag="rta")
                nc.vector.memset(rta[H : H + 1, :], 1.0)
                nc.vector.tensor_copy(rta[0:H, :], pt[:])
                po = psumhp.tile([128, cfg.C], f32, tag="po")
                nc.tensor.matmul(po[:], rta[:], m2w_t[:])
                nc.vector.tensor_copy(out_sb[:, b, :], po[:])

            outv = out_d[:].rearrange("(b p) f -> p b f", p=128)
            nc.sync.dma_start(out=outv, in_=out_sb[:])

    nc.finalize()

    # The program is immutable from here on; memoize its JSON serialization
    # on this instance (the jax lowering re-serializes the BIR every call).
    orig_to_json_bytes = nc.to_json_bytes
    cache = []

    def _memo_to_json_bytes():
        if not cache:
            cache.append(orig_to_json_bytes())
        return cache[0]

    nc.to_json_bytes = _memo_to_json_bytes
    return nc


# ------------------------------------------------------------ entry point
def kernel(**inputs) -> np.ndarray:
    cfg = Cfg()
    rcoef = horner_coeffs(cfg)
    plan = build_plan(cfg, inputs)
    nc = build_program(cfg, plan, rcoef)

    from concourse.bass_utils import run_bass_kernel_spmd

    res = run_bass_kernel_spmd(nc, plan.in_maps, list(range(cfg.NCORES)))
    out = np.concatenate(
        [res.results[c]["outp"][: cfg.NSH] for c in range(cfg.NCORES)], axis=0
    )
    return out.astype(np.float32)


# revision 41
# speedup vs baseline: 1.0254x; 1.0022x over previous
"""Trainium2 Bass kernel for CGNN message-passing ODE (nn_CGNN_51333449121989).

Math: the reference integrates the affine ODE z' = diag(sigmoid(alpha))*0.5*(A z - z) + x0
with RK4 (4 steps, dt=0.25) from z0 = x0, where x0 = [x @ m1_w + m1_b, zeros].
Since each RK4 step is the affine map z <- P(M) z + Q(M) x0 with
M = diag(a)*0.5*(A - I), the final state is an exact degree-16 polynomial
R(M) x0, evaluated here by 16 Horner iterations:
    y <- a05 * (A y - y) + r_k * x0      (a05 = 0.5*sigmoid(alpha))
Feature columns H..2H-1 of the state are identically zero (columns evolve
independently and start/force at zero), so the working state is [N, H].

Distribution: 1D node partition over 8 cores (6250 rows each, padded to
6272 = 49*128).  Each core owns the edges whose src falls in its row range.
Per iteration each core:
  - dma_gather's y[dst] rows (256B each) from a full HBM replica of y
    (int16 gather indices => the replica is split in two halves; edges are
    routed into a "low" and a "high" gather stream),
  - segment-sums messages into psum per 128-row block with PE matmuls:
    psum[128 rows, H] += onehot(src_local)^T @ msg, where the per-chunk
    onehot-with-weights matrix W[e, r] = (r == src_local[e]) * w_e is built
    on the vector engine from an iota tile via a dual-op tensor_scalar,
  - applies the Horner update, publishes its shard and AllGathers the next
    replica.
The 16 iterations are flat python-unrolled: collectives cannot execute
inside a For_i hardware loop on this toolchain, and dma_scatter_add loses
colliding updates, so the onehot-matmul segment-sum stays.

Transfer layer: the per-call wall time is dominated by host->device upload
through the PJRT tunnel (~40MB/s for incompressible bytes, large per-array
fixed cost).  Countermeasures:
  - ONE flat int16 blob per core holds every input (9-bit fixed-point x
    unpacked on device via integer shift/mask DVE ops; bf16 tables; int8
    src rows and edge weights; int16 gather indices uploaded once at 16 rows and replicated
    to 128 on device; iota/identity built on device), output returned as
    bf16;
  - jax persistent compilation cache so warm calls skip the walrus backend
    compile, plus a per-instance memo of the BIR JSON serialization that
    the jax lowering otherwise recomputes every call.
Host-side work is limited to sharding/packing: edge bucketing + padding to a
core-uniform chunk structure, gather-index wrapping, and input layout.
"""

import os
import sys

sys.path.insert(0, "/opt/trn_rl_repo")

from dataclasses import dataclass

import numpy as np
import ml_dtypes


def _setup_jax_compilation_cache():
    """Persistent XLA compilation cache: warm run_bass_kernel_spmd calls hit
    the cache instead of re-running the (~2s) walrus backend compile."""
    try:
        import jax

        jax.config.update("jax_compilation_cache_dir", "/tmp/jax_comp_cache")
        jax.config.update("jax_persistent_cache_min_compile_time_secs", 0)
        jax.config.update("jax_persistent_cache_min_entry_size_bytes", -1)
    except Exception:
        pass


_setup_jax_compilation_cache()


# ---------------------------------------------------------------- constants
@dataclass(frozen=True)
class Cfg:
    N: int = 50000          # nodes
    E: int = 600000         # edges
    F: int = 500            # input features
    H: int = 64             # hidden (ODE state width)
    C: int = 40             # classes
    NCORES: int = 8
    NITER: int = 16         # Horner iterations (degree-16 polynomial, exact)
    DT: float = 0.25        # T / STEPS from the reference
    GCH: int = 48           # gather-group size in chunks (48*128 idx per call)

    @property
    def NSH(self):          # true rows per core
        return self.N // self.NCORES

    @property
    def BLOCKS(self):       # 128-row blocks per core
        return (self.NSH + 127) // 128

    @property
    def NLOC(self):         # padded rows per core
        return self.BLOCKS * 128

    @property
    def NREP(self):         # replica rows
        return self.NCORES * self.NLOC

    @property
    def HALF(self):         # low/high split of replica rows (int16 gather idx)
        return self.NREP // 2

    @property
    def KP(self):           # encoder contraction dim padded (F + bias row)
        return ((self.F + 1 + 127) // 128) * 128


def horner_coeffs(cfg: Cfg) -> np.ndarray:
    """Coefficients r_0..r_16 of the exact RK4 polynomial R(M)."""
    dt = cfg.DT
    deg = max(cfg.NITER, 16)
    P = np.zeros(deg + 1)
    Q = np.zeros(deg + 1)
    P[0] = 1.0
    fact = 1.0
    for j in range(1, 5):
        fact *= j
        P[j] = dt**j / fact
        Q[j - 1] = dt**j / fact

    def pmul(a, b):
        out = np.zeros(2 * deg + 1)
        for i in range(deg + 1):
            if a[i]:
                out[i : i + deg + 1] += a[i] * b
        return out[: deg + 1]

    P2 = pmul(P, P)
    P3 = pmul(P2, P)
    P4 = pmul(P3, P)
    S = P3 + P2 + P
    S[0] += 1.0
    R = P4 + pmul(S, Q)
    return R


# ------------------------------------------------------------ tile patch
def _patch_tile_drain():
    """This toolchain's walrus rejects instructions with several sem waits;
    split TileContext's exit-drain waits across single-wait nops."""
    import concourse.tile as tile
    from concourse.vector_clock import ScopedClock
    from bass_rust import VectorClock

    if getattr(tile.TileContext, "_drain_patched", False):
        return

    def _drain_and_barrier(self, tick_clock, wait_clock):
        gc = tick_clock.global_clock
        scoped = ScopedClock({None: gc})
        for scope, vc in scoped.items():
            procs = [i for i in range(len(vc)) if vc[i] > 0]
            for p in procs:
                pvc = VectorClock()
                pvc.require_at_least(p, vc[p])
                nop = self.nc.sync.nop(nofuse=True, hint="drain_split")
                wait_clock.add_sem_waits(nop.ins, ScopedClock({scope: pvc}))
        self.nc.sync.drain()
        self.nc.all_engine_barrier()
        assert self.sems is not None
        popped = self.nc._tile_sem_poison_stack.pop()
        assert popped is self._sem_poison
        self.nc.clear_and_free_semaphores(list(self.sems.allocated().values()))
        self.nc.all_engine_barrier()

    tile.TileContext._drain_and_barrier = _drain_and_barrier
    tile.TileContext._drain_patched = True


# ------------------------------------------------------------ host prep
def _align(v, a=128):
    return (v + a - 1) // a * a


@dataclass
class Plan:
    # uniform chunk structure
    nch_l: int
    nch_h: int
    cbs: np.ndarray           # [BLOCKS, 2] chunks per (block, stream)
    ngrp_l: int
    ngrp_h: int
    # 9-bit x quantization scale (global)
    xscale: float
    # blob section offsets (int16 elements), core-uniform
    off: dict
    blob_len: int
    # per-core packed tensors
    in_maps: list


def build_plan(cfg: Cfg, inputs: dict) -> Plan:
    x = np.asarray(inputs["x"], np.float32)
    ew = np.asarray(inputs["edge_w"], np.float32)
    src = np.asarray(inputs["edge_src"], np.int64)
    dst = np.asarray(inputs["edge_dst"], np.int64)
    m1w = np.asarray(inputs["m1_w"], np.float32)
    m1b = np.asarray(inputs["m1_b"], np.float32)
    alpha = np.asarray(inputs["alpha_train"], np.float32)
    m2w = np.asarray(inputs["m2_w"], np.float32)
    m2b = np.asarray(inputs["m2_b"], np.float32)
    bf16 = ml_dtypes.bfloat16

    NC, NSH, NLOC, BLOCKS = cfg.NCORES, cfg.NSH, cfg.NLOC, cfg.BLOCKS
    HALF, GCH, KP = cfg.HALF, cfg.GCH, cfg.KP

    owner = src // NSH
    owner = np.minimum(owner, NC - 1)
    src_loc = src - owner * NSH
    downer = dst // NSH
    downer = np.minimum(downer, NC - 1)
    dpos = downer * NLOC + (dst - downer * NSH)   # replica row of dst
    stream = (dpos >= HALF).astype(np.int64)      # 0 = low, 1 = high
    block = src_loc // 128
    srow = src_loc % 128                          # row within block

    # ---- per-(core, block, stream) edge buckets
    counts = np.zeros((NC, BLOCKS, 2), np.int64)
    np.add.at(counts, (owner, block, stream), 1)
    cbs = np.ceil(counts.max(axis=0) / 128).astype(np.int64)   # [BLOCKS, 2]
    cbs[:, 0] = np.maximum(cbs[:, 0], 1)    # every block needs >=1 chunk
    nch_l = int(cbs[:, 0].sum())
    nch_h = int(cbs[:, 1].sum())
    nch = nch_l + nch_h
    ngrp_l = (nch_l + GCH - 1) // GCH
    ngrp_h = (nch_h + GCH - 1) // GCH
    nil = ngrp_l * GCH * 8        # idx columns per 16-row band, low
    nih = ngrp_h * GCH * 8

    # chunk column offsets per (block, stream); stream H columns offset by nch_l
    off_l = np.concatenate([[0], np.cumsum(cbs[:, 0])])
    off_h = np.concatenate([[0], np.cumsum(cbs[:, 1])])

    # ---- 9-bit fixed point for x: q = round(x*s) in [-255, 255]
    xscale = 255.0 / max(float(np.abs(x).max()), 1e-6)

    # ---- blob layout (int16 elements, 128-aligned sections, core-uniform)
    off = {}
    pos = 0
    def sect(name, n):
        nonlocal pos
        off[name] = pos
        pos += _align(n)
    sect("xtb9", BLOCKS * KP * 72)          # packed 9-bit [BLOCKS, KP, 9, 8]
    sect("m1w", KP * cfg.H)                 # bf16 [KP, H]
    sect("m2w", (cfg.H + 1) * cfg.C * 2)    # f32  [H+1, C]
    sect("alpha", 128 * BLOCKS)             # bf16 [128, BLOCKS]
    sect("srct", 64 * nch)                  # int8 [128, nch]
    sect("wt", 64 * nch)                    # int8 [128, nch], (v+128)/255
    sect("idxl", 16 * nil)                  # int16 [16, nil]
    sect("idxh", 16 * nih)                  # int16 [16, nih]
    blob_len = _align(pos, 1024)

    m1w_aug = np.zeros((KP, cfg.H), np.float32)
    m1w_aug[: cfg.F] = m1w
    m1w_aug[cfg.F] = m1b
    m1w_b = m1w_aug.astype(bf16)
    m2w_aug = np.zeros((cfg.H + 1, cfg.C), np.float32)
    m2w_aug[: cfg.H] = m2w
    m2w_aug[cfg.H] = m2b

    def put(blob, name, arr):
        a16 = np.ascontiguousarray(arr).reshape(-1).view(np.int16)
        blob[off[name] : off[name] + a16.size] = a16

    def pack9(vals):
        """vals [R, 128] signed ints in [-256, 255] -> [R, 9, 8] uint16."""
        q = (vals.astype(np.int32) + 256).astype(np.uint32).reshape(-1, 8, 16)
        qq = [q[:, :, i] for i in range(16)]
        w = [
            (qq[0] << 7) | (qq[1] >> 2),
            ((qq[1] & 0x3) << 14) | (qq[2] << 5) | (qq[3] >> 4),
            ((qq[3] & 0xF) << 12) | (qq[4] << 3) | (qq[5] >> 6),
            ((qq[5] & 0x3F) << 10) | (qq[6] << 1) | (qq[7] >> 8),
            ((qq[7] & 0xFF) << 8) | (qq[8] >> 1),
            ((qq[8] & 0x1) << 15) | (qq[9] << 6) | (qq[10] >> 3),
            ((qq[10] & 0x7) << 13) | (qq[11] << 4) | (qq[12] >> 5),
            ((qq[12] & 0x1F) << 11) | (qq[13] << 2) | (qq[14] >> 7),
            ((qq[14] & 0x7F) << 9) | qq[15],
        ]
        return np.stack(w, axis=1).astype(np.uint16)   # [R, 9, 8]

    # sort edges per core by (stream, block, dst) for gather locality
    in_maps = []
    for c in range(NC):
        sel = owner == c
        eb, es, er, ed, ewc = (
            block[sel], stream[sel], srow[sel], dpos[sel], ew[sel])

        src_tab = np.zeros((128, nch), np.int8)
        w_tab = np.full((128, nch), -128, np.int8)   # -128 -> w = 0.0
        idx_l = np.zeros(nch_l * 128, np.int64)
        idx_h = np.zeros(nch_h * 128, np.int64)

        for s, (idx_arr, offc, base_col) in enumerate(
            [(idx_l, off_l, 0), (idx_h, off_h, nch_l)]
        ):
            m = es == s
            b_s, r_s, d_s, w_s = eb[m], er[m], ed[m], ewc[m]
            order = np.lexsort((d_s, b_s))
            b_s, r_s, d_s, w_s = b_s[order], r_s[order], d_s[order], w_s[order]
            # place edges of block b into its chunk range [offc[b], offc[b+1])
            starts = np.searchsorted(b_s, np.arange(BLOCKS))
            ends = np.searchsorted(b_s, np.arange(BLOCKS), side="right")
            for b in range(BLOCKS):
                n_edges = ends[b] - starts[b]
                pos0 = offc[b] * 128
                sl = slice(starts[b], ends[b])
                idx_arr[pos0 : pos0 + n_edges] = d_s[sl] - s * HALF
                cols = np.arange(n_edges) // 128 + offc[b]
                rows = np.arange(n_edges) % 128
                src_tab[rows, base_col + cols] = r_s[sl].astype(np.int8)
                w_tab[rows, base_col + cols] = (
                    np.round(w_s[sl] * 255.0).astype(np.int32) - 128
                ).astype(np.int8)
                # padding edges keep w=0 / idx=0 / src_row=0

        def wrap_idx(idx_arr, ngrp):
            n = ngrp * GCH * 128
            full = np.zeros(n, np.int64)
            full[: len(idx_arr)] = idx_arr
            return full.reshape(-1, 16).T.astype(np.int16)    # [16, n/16]

        # encoder inputs: block-tiled x^T with bias row, 10-bit packed
        rows = slice(c * NSH, (c + 1) * NSH)
        xsh = np.zeros((NLOC, cfg.F), np.float32)
        xsh[:NSH] = x[rows]
        xtb_q = np.zeros((BLOCKS, KP, 128), np.int32)
        for b in range(BLOCKS):
            xtb_q[b, : cfg.F, :] = np.round(
                xsh[b * 128 : (b + 1) * 128].T * xscale).astype(np.int32)
            xtb_q[b, cfg.F, :] = int(round(xscale))     # bias row ~ 1.0
        xtb_p = pack9(xtb_q.reshape(-1, 128)).reshape(BLOCKS, KP, 9, 8)

        al = np.zeros(NLOC, np.float32)
        al[:NSH] = alpha[rows]
        alpha_b = al.reshape(BLOCKS, 128).T.astype(bf16)    # [128, BLOCKS]

        blob = np.zeros(blob_len, np.int16)
        put(blob, "xtb9", xtb_p)
        put(blob, "m1w", m1w_b)
        put(blob, "m2w", m2w_aug)
        put(blob, "alpha", alpha_b)
        put(blob, "srct", src_tab)
        put(blob, "wt", w_tab)
        put(blob, "idxl", wrap_idx(idx_l, ngrp_l))
        put(blob, "idxh", wrap_idx(idx_h, ngrp_h))
        in_maps.append(dict(blob=blob))

    return Plan(nch_l, nch_h, np.asarray(cbs), ngrp_l, ngrp_h,
                xscale, off, blob_len, in_maps)


# ------------------------------------------------------------ device program
def build_program(cfg: Cfg, plan: Plan, rcoef: np.ndarray,
                  timing_mode: bool = False, phases: str = "ehda",
                  reps=(1, 1, 1)):
    RG, RM, RU = reps   # timing: repeat gathers / matmuls / updates
    """timing_mode: single-core variant for TimelineSim (collectives replaced
    by a local DMA of the same local traffic)."""
    import concourse.bacc as bacc
    import concourse.mybir as mybir
    import concourse.tile as tile

    _patch_tile_drain()

    NC, H, BLOCKS, NLOC, NREP, HALF = (
        cfg.NCORES, cfg.H, cfg.BLOCKS, cfg.NLOC, cfg.NREP, cfg.HALF)
    GCH, KP = cfg.GCH, cfg.KP
    KCH = KP // 128
    f32 = mybir.dt.float32
    bf16 = mybir.dt.bfloat16
    i16 = mybir.dt.int16
    i8 = mybir.dt.int8
    Alu = mybir.AluOpType
    nch = plan.nch_l + plan.nch_h
    nil = plan.ngrp_l * GCH * 8
    nih = plan.ngrp_h * GCH * 8
    O = plan.off

    nc = bacc.Bacc("TRN2", target_bir_lowering=False, debug=False,
                   num_devices=1 if timing_mode else NC)

    def allgather(ins, outs):
        if "a" not in phases:
            return
        if timing_mode:
            # local-cost stand-in: write own shard into the replica
            nc.sync.dma_start(out=outs[0][0:NLOC, :], in_=ins[0])
            return
        nc.gpsimd.collective_compute(
            "AllGather", mybir.AluOpType.bypass,
            replica_groups=[list(range(NC))], ins=ins, outs=outs,
        )

    blob_d = nc.dram_tensor("blob", [plan.blob_len], i16, kind="ExternalInput")
    out_d = nc.dram_tensor("outp", [NLOC, cfg.C], bf16, kind="ExternalOutput")

    def bview(name, n, dt, p):
        """[p, n/p] view of blob section `name` with dtype dt (n in dt elems)."""
        n16 = {f32: 2 * n, i8: n // 2}.get(dt, n)
        v = blob_d[O[name] : O[name] + n16]
        if dt != i16:
            v = v.bitcast(dt)
        return v.rearrange("(p f) -> p f", p=p)

    ag_in = nc.dram_tensor("ag_in", [NLOC, H], f32)
    rep = [
        nc.dram_tensor(f"rep{j}", [NREP, H], f32, addr_space="Shared")
        for j in range(2)
    ]

    R = [float(v) for v in rcoef]
    off_l = np.concatenate([[0], np.cumsum(plan.cbs[:, 0])]).astype(int)
    off_h = np.concatenate([[0], np.cumsum(plan.cbs[:, 1])]).astype(int)

    with tile.TileContext(nc) as tc:
        with (
            tc.tile_pool(name="const", bufs=1) as constp,
            tc.tile_pool(name="xin", bufs=4) as xinp,
            tc.tile_pool(name="xun", bufs=3) as xunp,
            tc.tile_pool(name="msgl", bufs=2) as msglp,
            tc.tile_pool(name="msgh", bufs=2) as msghp,
            tc.tile_pool(name="msglb", bufs=2) as msglbp,
            tc.tile_pool(name="msghb", bufs=2) as msghbp,
            tc.tile_pool(name="wones", bufs=4) as wp,
            tc.tile_pool(name="upd", bufs=4) as updp,
            tc.tile_pool(name="head", bufs=3) as headp,
            tc.tile_pool(name="psum", bufs=4, space="PSUM") as psump,
            tc.tile_pool(name="psumh", bufs=2, space="PSUM") as psumhp,
        ):
            # ---------- resident tiles
            iota16_t = constp.tile([128, 128], i16)
            iota_t = constp.tile([128, 128], bf16)
            iotaf_t = constp.tile([128, 128], f32)
            pid16_t = constp.tile([128, 1], i16)
            pidf_t = constp.tile([128, 1], f32)
            ident_t = constp.tile([128, 128], f32)
            srct_t = constp.tile([128, nch], f32)
            wt_t = constp.tile([128, nch], f32)
            srct8_t = constp.tile([128, nch], i8)
            wt8_t = constp.tile([128, nch], i8)
            idxl_t = constp.tile([128, nil], i16)
            idxh_t = constp.tile([128, nih], i16)
            m2w_t = constp.tile([cfg.H + 1, cfg.C], f32)
            alpha_t = constp.tile([128, BLOCKS], bf16)
            a05_t = constp.tile([128, BLOCKS], f32)
            x0_t = constp.tile([128, BLOCKS, H], f32)
            y_t = constp.tile([128, BLOCKS, H], f32)
            out_sb = constp.tile([128, BLOCKS, cfg.C], bf16)

            for t, name, n, dt, p in [
                (srct8_t, "srct", 128 * nch, i8, 128),
                (wt8_t, "wt", 128 * nch, i8, 128),
                (m2w_t, "m2w", (cfg.H + 1) * cfg.C, f32, cfg.H + 1),
                (alpha_t, "alpha", 128 * BLOCKS, bf16, 128),
            ]:
                nc.sync.dma_start(out=t[:], in_=bview(name, n, dt, p))
            # iota / partition-id / identity built on device
            nc.gpsimd.iota(iota16_t[:], [[1, 128]], channel_multiplier=0)
            nc.gpsimd.iota(pid16_t[:], [[0, 1]], channel_multiplier=1)
            nc.vector.tensor_copy(iota_t[:], iota16_t[:])
            nc.vector.tensor_copy(iotaf_t[:], iota16_t[:])
            nc.vector.tensor_copy(pidf_t[:], pid16_t[:])
            nc.vector.tensor_scalar(ident_t[:], iotaf_t[:], pidf_t[:], None,
                                    Alu.is_equal)
            # gather indices: load the 16-row band 8x from DRAM to fill 128
            for t, name, w in [(idxl_t, "idxl", nil), (idxh_t, "idxh", nih)]:
                for k in range(8):
                    nc.sync.dma_start(out=t[16 * k : 16 * (k + 1), :],
                                      in_=bview(name, 16 * w, i16, 16))
            # m1w: KP > 128 partitions -> load as KCH separate [128, H] tiles
            m1w_ts = []
            for kc in range(KCH):
                mt = constp.tile([128, H], bf16, tag=f"m1w{kc}")
                nc.sync.dma_start(
                    out=mt[:],
                    in_=blob_d[O["m1w"] + kc * 128 * H : O["m1w"] + (kc + 1) * 128 * H]
                    .bitcast(bf16).rearrange("(p f) -> p f", p=128))
                m1w_ts.append(mt)

            nc.vector.tensor_copy(srct_t[:], srct8_t[:])
            # w = (v + 128) / 255
            nc.vector.tensor_scalar(wt_t[:], wt8_t[:], 128.0, 1.0 / 255.0,
                                    Alu.add, Alu.mult)
            nc.scalar.activation(a05_t[:], alpha_t[:],
                                 mybir.ActivationFunctionType.Sigmoid)
            nc.vector.tensor_scalar_mul(a05_t[:], a05_t[:], 0.5)

            # ---------- encoder: x0 = x @ m1_w + b ; y = r16 * x0
            inv_s = 1.0 / plan.xscale
            M9 = 0x1FF
            # stream spec: single-word  (wa, rshift, mask, None)
            #                -> v = (wa >> rshift) & mask
            #              dual-word    (wa, mask_a, lshift, (wb, rshift_b, mask_b))
            #                -> v = ((wa & mask_a) << lshift) | ((wb >> rshift_b) & mask_b)
            # (v15 = w8 & 0x1FF is special-cased below)
            UNPACK9 = [
                (0, 7, M9, None), (0, 0x7F, 2, (1, 14, 0x3)),
                (1, 5, M9, None), (1, 0x1F, 4, (2, 12, 0xF)),
                (2, 3, M9, None), (2, 0x7, 6, (3, 10, 0x3F)),
                (3, 1, M9, None), (3, 0x1, 8, (4, 8, 0xFF)),
                (4, 0xFF, 1, (5, 15, 0x1)),
                (5, 6, M9, None), (5, 0x3F, 3, (6, 13, 0x7)),
                (6, 4, M9, None), (6, 0xF, 5, (7, 11, 0x1F)),
                (7, 2, M9, None), (7, 0x3, 7, (8, 9, 0x7F)),
                (8, 0x1FF, 0, None),
            ]
            for b in range(BLOCKS if "e" in phases else 0):
                pe = psump.tile([128, H], f32, tag="acc")
                xq = xinp.tile([128, KCH, 9, 8], i16)
                nc.sync.dma_start(
                    out=xq[:],
                    in_=blob_d[O["xtb9"] + b * KP * 72 : O["xtb9"] + (b + 1) * KP * 72]
                    .rearrange("(kc p w g) -> p kc w g", p=128, w=9, g=8))
                xt = xunp.tile([128, KCH, 128], bf16, tag="xt")
                wv = [xq[:, :, j, :] for j in range(9)]
                va = xunp.tile([128, KCH, 8], i16, tag="va")
                vb = xunp.tile([128, KCH, 8], i16, tag="vb")
                ts = nc.vector.tensor_scalar
                for s, spec in enumerate(UNPACK9):
                    if spec[3] is None:
                        if s == 16 - 1:  # v15 = w8 & 0x1FF
                            ts(va[:], wv[8], M9, 0,
                               Alu.bitwise_and, Alu.logical_shift_right)
                        else:
                            wa, ra, ma, _ = spec
                            ts(va[:], wv[wa], ra, ma,
                               Alu.logical_shift_right, Alu.bitwise_and)
                        out_v = va
                    else:
                        wa, ma, la, (wb, rb, mb) = spec
                        ts(va[:], wv[wa], ma, la,
                           Alu.bitwise_and, Alu.logical_shift_left)
                        ts(vb[:], wv[wb], rb, mb,
                           Alu.logical_shift_right, Alu.bitwise_and)
                        nc.vector.tensor_tensor(va[:], va[:], vb[:],
                                                Alu.bitwise_or)
                        out_v = va
                    nc.scalar.activation(
                        xt[:, :, s::16], out_v[:],
                        mybir.ActivationFunctionType.Copy,
                        scale=inv_s, bias=-256.0 * inv_s)
                for kc in range(KCH):
                    nc.tensor.matmul(pe[:], xt[:, kc, :], m1w_ts[kc][:],
                                     start=(kc == 0), stop=(kc == KCH - 1))
                nc.scalar.activation(x0_t[:, b, :], pe[:],
                                     mybir.ActivationFunctionType.Copy)
                nc.vector.tensor_scalar_mul(y_t[:, b, :], pe[:], R[cfg.NITER])

            # publish y -> replica 0
            agv = ag_in[:].rearrange("(b p) f -> p b f", p=128)
            nc.sync.dma_start(out=agv, in_=y_t[:])
            allgather([ag_in[:]], [rep[0][:]])

            # ---------- Horner iterations (flat; collectives cannot sit in
            # a For_i loop on this toolchain)
            nidx_reg = nc.gpsimd.to_reg(GCH * 128)

            def horner_iter(src_rep, dst_rep, rk_imm, last):
                # gathers for both streams; convert messages to bf16
                msg_tiles = {0: [], 1: []}
                for s, (pool, poolb, idx_t, ngrp, base) in enumerate([
                    (msglp, msglbp, idxl_t, plan.ngrp_l, 0),
                    (msghp, msghbp, idxh_t, plan.ngrp_h, HALF),
                ]):
                    view = src_rep[base : base + HALF, :]
                    for g in range(ngrp):
                        mt = pool.tile([128, GCH, H], f32, tag=f"msg{s}")
                        for _ in range(RG):
                            nc.gpsimd.dma_gather(
                                mt[:], view,
                                idx_t[:, g * GCH * 8 : (g + 1) * GCH * 8],
                                GCH * 128, nidx_reg, H, elem_step=H,
                                single_packet=False)
                        mtb = poolb.tile([128, GCH, H], bf16, tag=f"msgb{s}")
                        nc.vector.tensor_copy(mtb[:], mt[:])
                        msg_tiles[s].append(mtb)

                for b in range(BLOCKS):
                    ps = psump.tile([128, H], f32, tag="acc")
                    ncl = int(plan.cbs[b, 0])
                    nchh = int(plan.cbs[b, 1])
                    tot = ncl + nchh
                    for rm in range(RM):
                        ci = 0
                        for s, n_s, offc in ((0, ncl, off_l), (1, nchh, off_h)):
                            for j in range(n_s):
                                col = offc[b] + j            # stream-chunk idx
                                tabcol = col + (plan.nch_l if s else 0)
                                wt_tile = wp.tile([128, 128], bf16, tag="W")
                                nc.vector.tensor_scalar(
                                    wt_tile[:], iota_t[:],
                                    srct_t[:, tabcol : tabcol + 1],
                                    wt_t[:, tabcol : tabcol + 1],
                                    Alu.is_equal, Alu.mult)
                                mtb = msg_tiles[s][col // GCH]
                                nc.tensor.matmul(
                                    ps[:], wt_tile[:], mtb[:, col % GCH, :],
                                    start=(ci == 0 and rm == 0),
                                    stop=(ci == tot - 1 and rm == RM - 1),
                                    skip_group_check=True)
                                ci += 1
                    # y' = a05*(az - y) + r_k*x0  == a05*az - (a05*y - r_k*x0)
                    for ru in range(RU):
                        x0s = updp.tile([128, H], f32, tag="x0s")
                        nc.scalar.activation(
                            x0s[:], x0_t[:, b, :],
                            mybir.ActivationFunctionType.Copy, scale=rk_imm)
                        tt = updp.tile([128, H], f32, tag="tt")
                        nc.vector.scalar_tensor_tensor(
                            tt[:], y_t[:, b, :], a05_t[:, b : b + 1], x0s[:],
                            Alu.mult, Alu.subtract)
                        nc.vector.scalar_tensor_tensor(
                            y_t[:, b, :], ps[:], a05_t[:, b : b + 1], tt[:],
                            Alu.mult, Alu.subtract)

                if not last:
                    # publish y for the next iteration
                    nc.sync.dma_start(out=agv, in_=y_t[:])
                    allgather([ag_in[:]], [dst_rep[:]])

            for i in range(cfg.NITER if "h" in phases else 0):
                horner_iter(rep[i % 2], rep[(i + 1) % 2],
                            R[cfg.NITER - 1 - i], i == cfg.NITER - 1)

            # ---------- head: out = relu(y) @ m2_w + b  (all f32)
            for b in range(BLOCKS if "d" in phases else 0):
                rt = headp.tile([128, H], f32, tag="relu")
                nc.scalar.activation(rt[:], y_t[:, b, :],
                                     mybir.ActivationFunctionType.Relu)
                pt = psumhp.tile([H, 128], f32, tag="tp")
                nc.tensor.transpose(pt[:], rt[:], ident_t[:])
                rta = headp.tile([H + 1, 128], f32, tag="rta")
                nc.vector.memset(rta[H : H + 1, :], 1.0)
                nc.vector.tensor_copy(rta[0:H, :], pt[:])
                po = psumhp.tile([128, cfg.C], f32, tag="po")
                nc.tensor.matmul(po[:], rta[:], m2w_t[:])
                nc.vector.tensor_copy(out_sb[:, b, :], po[:])

            outv = out_d[:].rearrange("(b p) f -> p b f", p=128)
            nc.sync.dma_start(out=outv, in_=out_sb[:])

    nc.finalize()

    # The program is immutable from here on; memoize its JSON serialization
    # on this instance (the jax lowering re-serializes the BIR every call).
    orig_to_json_bytes = nc.to_json_bytes
    cache = []

    def _memo_to_json_bytes():
        if not cache:
            cache.append(orig_to_json_bytes())
        return cache[0]

    nc.to_json_bytes = _memo_to_json_bytes
    return nc


# ------------------------------------------------------------ entry point
def kernel(**inputs) -> np.ndarray:
    cfg = Cfg()
    rcoef = horner_coeffs(cfg)
    plan = build_plan(cfg, inputs)
    nc = build_program(cfg, plan, rcoef)

    from concourse.bass_utils import run_bass_kernel_spmd

    res = run_bass_kernel_spmd(nc, plan.in_maps, list(range(cfg.NCORES)))
    out = np.concatenate(
        [res.results[c]["outp"][: cfg.NSH] for c in range(cfg.NCORES)], axis=0
    )
    return out.astype(np.float32)
